# revision 2
# baseline (speedup 1.0000x reference)
"""BayesianGNN forward pass on 8 Trainium2 NeuronCores (Bass/Tile).

The reference network's jraph aggregations are dead code, so nodes/edges/
globals are independent per-row MLP chains.  The Bayesian weight noise is
deterministic (EPS_SEED=42), so effective dense weights are computed on the
host and adjacent layer pairs with no ReLU between them are folded:

  node/edge chain: x[64] -> (We@W00) relu -> W01 relu -> (W02@W10) relu
                   -> W11 relu -> W12                      (5 matmuls/row)
  globals chain:   g[64] -> 6x[128,128] relu -> readout [128,1]  (N=16)

Device layout: activations feature-major [feat, rows]; rows data-parallel
over 8 cores; each 512-row half-chunk is an independent pipeline:
matmul (float32r) -> PSUM [128,512] -> fused bias(+relu) epilogue
(ScalarE/VectorE 7:6 weighted round-robin) -> SBUF -> DMA out.
"""
import os
import sys

if '/opt/trn_rl_repo' not in sys.path:
    sys.path.insert(0, '/opt/trn_rl_repo')

import numpy as np

import concourse.tile as tile
from concourse import bacc, mybir, bass_utils

f32 = mybir.dt.float32
f32r = mybir.dt.float32r
AF = mybir.ActivationFunctionType
ALU = mybir.AluOpType

N_NODES = 100_000
N_EDGES = 400_000
N_CORES = 8
PN = N_NODES // N_CORES            # node rows per core
PEDG = N_EDGES // N_CORES          # edge rows per core
DCH = 1024
RN = ((PN + DCH - 1) // DCH) * DCH     # 13312
RE = ((PEDG + DCH - 1) // DCH) * DCH   # 50176
NDCH_N = RN // DCH
NDCH_E = RE // DCH

MODE = os.environ.get("GNN_MODE", "f32r")   # "f32r" | "f32"


# ----------------------------------------------------------------- weights
def _pick_eps_backend(nodes):
    """The environment's default PRNG impl (rbg) is platform-dependent.
    The reference generated its inputs and weight noise on some backend;
    find a backend whose rbg stream reproduces `nodes` so our eps matches
    the reference's."""
    import jax

    probe = np.asarray(nodes[:2], np.float32)

    def gen(dev):
        with jax.default_device(dev):
            ks = jax.random.split(jax.random.key(0), 12)
            cand = jax.random.normal(ks[0], (N_NODES, 64), 'float32')
            return np.asarray(cand[:2], np.float32)

    default_dev = jax.devices()[0]
    try:
        if np.allclose(gen(default_dev), probe, atol=1e-6):
            return default_dev
    except Exception:
        pass
    try:
        cpu = jax.devices('cpu')[0]
        if np.allclose(gen(cpu), probe, atol=1e-6):
            return cpu
    except Exception:
        pass
    return default_dev


def fold_params(params, nodes):
    import jax
    import jax.numpy as jnp

    dev = _pick_eps_backend(nodes)

    with jax.default_device(dev):
        def eff_layers(layers, key):
            ks = jax.random.split(key, len(layers))
            out = []
            for lyr, k in zip(layers, ks):
                wm, wr = jnp.asarray(lyr['wm']), jnp.asarray(lyr['wr'])
                bm, br = jnp.asarray(lyr['bm']), jnp.asarray(lyr['br'])
                w = wm + jax.random.normal(k, wm.shape) * jnp.log1p(jnp.exp(wr))
                b = bm + jax.random.normal(k, bm.shape) * jnp.log1p(jnp.exp(br))
                out.append((np.asarray(w, np.float64),
                            np.asarray(b, np.float64)))
            return out

        keys = jax.random.split(jax.random.key(42), 4)
        n_mlps = [eff_layers(params['node_mlp'][i], keys[i]) for i in range(2)]
        e_mlps = [eff_layers(params['edge_mlp'][i], keys[i]) for i in range(2)]
        g_mlps = [eff_layers(params['global_mlp'][i], keys[i])
                  for i in range(2)]
        ro = eff_layers(params['readout'], keys[3])

    def dense(p):
        return (np.asarray(p['w'], np.float64), np.asarray(p['b'], np.float64))

    We, be = dense(params['node_embed'])
    Ee, ee = dense(params['edge_embed'])
    Ge, ge = dense(params['global_embed'])

    def chain(embed_w, embed_b, mlps):
        (w00, b00), (w01, b01), (w02, b02) = mlps[0]
        (w10, b10), (w11, b11), (w12, b12) = mlps[1]
        A = [embed_w @ w00, w01, w02 @ w10, w11, w12]
        c = [embed_b @ w00 + b00, b01, b02 @ w10 + b10, b11, b12]
        return A, c

    An, cn = chain(We, be, n_mlps)
    Ae, ce = chain(Ee, ee, e_mlps)
    (gw00, gb00), (gw01, gb01), (gw02, gb02) = g_mlps[0]
    (gw10, gb10), (gw11, gb11), (gw12, gb12) = g_mlps[1]
    (r0, rb0), (r1, rb1), (r2, rb2) = ro
    Ag = [Ge @ gw00, gw01, gw02 @ gw10, gw11, gw12 @ r0, r1, r2]
    cg = [ge @ gw00 + gb00, gb01, gb02 @ gw10 + gb10, gb11,
          gb12 @ r0 + rb0, rb1, rb2]

    tof = lambda a: np.ascontiguousarray(np.asarray(a, np.float32))
    return {
        'An': [tof(a) for a in An], 'cn': [tof(a) for a in cn],
        'Ae': [tof(a) for a in Ae], 'ce': [tof(a) for a in ce],
        'Ag': [tof(a) for a in Ag], 'cg': [tof(a) for a in cg],
    }


# ----------------------------------------------------------------- program
_PROG = {}


def build_program(mode):
    if mode in _PROG:
        return _PROG[mode]
    mm_dt = f32r if mode == "f32r" else f32

    nc = bacc.Bacc("TRN2", debug=False)
    nodes_in = nc.dram_tensor("nodes_in", [128, RN // 2], mm_dt,
                              kind="ExternalInput").ap()
    edges_in = nc.dram_tensor("edges_in", [128, RE // 2], mm_dt,
                              kind="ExternalInput").ap()
    g_in = nc.dram_tensor("g_in", [64, 16], f32, kind="ExternalInput").ap()
    nodes_out = nc.dram_tensor("nodes_out", [128, RN], f32,
                               kind="ExternalOutput").ap()
    edges_out = nc.dram_tensor("edges_out", [128, RE], f32,
                               kind="ExternalOutput").ap()
    g_out = nc.dram_tensor("g_out", [1, 16], f32, kind="ExternalOutput").ap()

    wdr = {}
    for pre in ("n", "e"):
        wdr[pre] = {
            'w': [nc.dram_tensor(f"w{pre}{i}", [128, 128], mm_dt,
                                 kind="ExternalInput").ap() for i in range(5)],
            'b': [nc.dram_tensor(f"b{pre}{i}", [128, 1], f32,
                                 kind="ExternalInput").ap() for i in range(5)],
        }
    wg_dr, bg_dr = [], []
    for i in range(7):
        kshape = 64 if i == 0 else 128
        mshape = 1 if i == 6 else 128
        wg_dr.append(nc.dram_tensor(f"wg{i}", [kshape, mshape], f32,
                                    kind="ExternalInput").ap())
        bg_dr.append(nc.dram_tensor(f"bg{i}", [mshape, 1], f32,
                                    kind="ExternalInput").ap())

    with tile.TileContext(nc) as tc:
        with tc.tile_pool(name="wpool", bufs=1) as wpool, \
             tc.tile_pool(name="ipool", bufs=12) as ipool, \
             tc.tile_pool(name="hpool", bufs=32) as hpool, \
             tc.tile_pool(name="opool", bufs=8) as opool, \
             tc.tile_pool(name="pspool", bufs=8, space="PSUM") as pspool:

            wt, bt = {}, {}
            for pre in ("n", "e"):
                wt[pre], bt[pre] = [], []
                for i in range(5):
                    w = wpool.tile([128, 128], mm_dt, tag=f"w{pre}{i}",
                                   name=f"w{pre}{i}_sb")
                    nc.sync.dma_start(out=w[:], in_=wdr[pre]['w'][i])
                    wt[pre].append(w)
                    b = wpool.tile([128, 1], f32, tag=f"b{pre}{i}",
                                   name=f"b{pre}{i}_sb")
                    nc.sync.dma_start(out=b[:], in_=wdr[pre]['b'][i])
                    bt[pre].append(b)
            wgt, bgt = [], []
            for i in range(7):
                kshape = 64 if i == 0 else 128
                mshape = 1 if i == 6 else 128
                w = wpool.tile([kshape, mshape], f32, tag=f"wg{i}",
                               name=f"wg{i}_sb")
                nc.sync.dma_start(out=w[:], in_=wg_dr[i])
                wgt.append(w)
                b = wpool.tile([mshape, 1], f32, tag=f"bg{i}",
                               name=f"bg{i}_sb")
                nc.sync.dma_start(out=b[:], in_=bg_dr[i])
                bgt.append(b)

            # ---- node/edge main loops ----
            # Each 512-column half of a dchunk is an independent chain
            # (features on partitions).  Blocks of BLK dchunks = 2*BLK
            # half-chains, layer-major: the PE sees 2*BLK independent
            # matmuls per layer; [128,512] epilogues are distributed
            # ACT:DVE by a 7:6 weighted round-robin.
            BLK = 6
            EPAT = [0, 1, 0, 1, 0, 1, 0, 0, 1, 0, 1, 0, 1]  # 7 ACT, 6 DVE
            ectr = [0]

            def do_chain(x_in, x_out, W, B, ndch):
                for b0 in range(0, ndch, BLK):
                    blk = range(b0, min(b0 + BLK, ndch))
                    nhc = 2 * len(blk)
                    its = []
                    for d in blk:
                        it = ipool.tile([128, 512], mm_dt, tag="in",
                                        name=f"in{d}")
                        nc.sync.dma_start(out=it[:],
                                          in_=x_in[:, 512 * d:512 * (d + 1)])
                        its.append(it)
                    ots = [opool.tile([128, 1024], f32, tag="ot",
                                      name=f"ot{j}") for j in range(len(blk))]
                    cur = [None] * nhc
                    for L in range(5):
                        pss = [pspool.tile([128, 512], f32, tag="ps",
                                           name=f"ps_{L}_{q}")
                               for q in range(nhc)]
                        for q in range(nhc):
                            j, h = q // 2, q % 2
                            if L == 0:
                                lhsT = W[0][64 * h:64 * (h + 1), :]
                                rhs = its[j][64 * h:64 * (h + 1), :]
                            else:
                                lhsT = W[L][:]
                                rhs = cur[q][:]
                            nc.tensor.matmul(pss[q][:], lhsT, rhs,
                                             start=True, stop=True)
                        for q in range(nhc):
                            j, h = q // 2, q % 2
                            ps = pss[q]
                            on_act = EPAT[ectr[0] % len(EPAT)] == 0
                            ectr[0] += 1
                            if L < 4:
                                hb = hpool.tile([128, 512], mm_dt, tag="h",
                                                name=f"h_{L}_{q}")
                                if on_act:
                                    nc.scalar.activation(hb[:], ps[:],
                                                         AF.Relu,
                                                         bias=B[L][:])
                                else:
                                    nc.vector.tensor_scalar(
                                        out=hb[:], in0=ps[:], scalar1=B[L][:],
                                        scalar2=0.0, op0=ALU.add, op1=ALU.max)
                                cur[q] = hb
                            else:
                                dst = ots[j][:, 512 * h:512 * (h + 1)]
                                if on_act:
                                    nc.scalar.activation(dst, ps[:],
                                                         AF.Identity,
                                                         bias=B[4][:])
                                else:
                                    nc.vector.tensor_scalar_add(
                                        out=dst, in0=ps[:], scalar1=B[4][:])
                    for j, d in enumerate(blk):
                        nc.gpsimd.dma_start(
                            out=x_out[:, DCH * d:DCH * (d + 1)], in_=ots[j][:])

            do_chain(nodes_in, nodes_out, wt["n"], bt["n"], NDCH_N)
            do_chain(edges_in, edges_out, wt["e"], bt["e"], NDCH_E)

            # ---- globals chain (fp32, N=16, replicated on every core) ----
            gt = wpool.tile([64, 16], f32, tag="gt", name="gt_sb")
            nc.sync.dma_start(out=gt[:], in_=g_in)
            cur = gt
            for i in range(7):
                mshape = 1 if i == 6 else 128
                ps = pspool.tile([mshape, 16], f32, tag="ps",
                                 name=f"gps{i}")
                nc.tensor.matmul(ps[:], wgt[i][:], cur[:], start=True,
                                 stop=True)
                if i < 6:
                    nxt = wpool.tile([128, 16], f32, tag=f"gs{i % 2}",
                                     name=f"gs{i}")
                    nc.scalar.activation(nxt[:], ps[:], AF.Relu,
                                         bias=bgt[i][:])
                else:
                    nxt = wpool.tile([1, 16], f32, tag="gout_sb",
                                     name="gout_sb")
                    nc.scalar.activation(nxt[:], ps[:], AF.Identity,
                                         bias=bgt[i][:])
                cur = nxt
            nc.gpsimd.dma_start(out=g_out, in_=cur[:])


    nc.compile()
    _PROG[mode] = nc
    return nc


# ------------------------------------------------------------------ kernel
def _pack(xT, R):
    """[64, R] feature-major -> [128, R//2] two-row-group packed layout."""
    d = R // DCH
    return np.ascontiguousarray(
        xT.reshape(64, d, 2, 512).transpose(2, 0, 1, 3).reshape(128, R // 2))


LAST_RESULT = {}


def kernel(nodes, edges, globals_, params, senders, receivers):
    mode = MODE
    nc = build_program(mode)

    nodes = np.asarray(nodes, np.float32)
    edges = np.asarray(edges, np.float32)
    globals_ = np.asarray(globals_, np.float32)

    fw = fold_params(params, nodes)

    weight_map = {"g_in": np.ascontiguousarray(globals_.T)}
    for pre, Akey, ckey in (("n", "An", "cn"), ("e", "Ae", "ce")):
        A, c = fw[Akey], fw[ckey]
        # embed lhsT duplicated so both 64-partition halves carry it
        weight_map[f"w{pre}0"] = np.ascontiguousarray(
            np.concatenate([A[0], A[0]], axis=0))
        for i in range(1, 5):
            weight_map[f"w{pre}{i}"] = A[i]
        for i in range(5):
            weight_map[f"b{pre}{i}"] = np.ascontiguousarray(
                c[i].reshape(-1, 1))
    for i in range(7):
        weight_map[f"wg{i}"] = fw['Ag'][i]
        weight_map[f"bg{i}"] = np.ascontiguousarray(fw['cg'][i].reshape(-1, 1))

    in_maps = []
    for cidx in range(N_CORES):
        nT = np.zeros((64, RN), np.float32)
        nT[:, :PN] = nodes[cidx * PN:(cidx + 1) * PN].T
        eT = np.zeros((64, RE), np.float32)
        eT[:, :PEDG] = edges[cidx * PEDG:(cidx + 1) * PEDG].T
        m = dict(weight_map)
        m["nodes_in"] = _pack(nT, RN)
        m["edges_in"] = _pack(eT, RE)
        in_maps.append(m)

    trace = os.environ.get("GNN_TRACE") == "1"
    res = None
    last_err = None
    for _attempt in range(3):
        try:
            res = bass_utils.run_bass_kernel_spmd(
                nc, in_maps, core_ids=list(range(N_CORES)), trace=trace)
            break
        except Exception as e:  # transient device errors observed; retry
            last_err = e
    if res is None:
        raise last_err
    LAST_RESULT["res"] = res

    n_parts, e_parts = [], []
    for cidx in range(N_CORES):
        r = res.results[cidx]
        n_parts.append(r["nodes_out"][:, :PN].T)
        e_parts.append(r["edges_out"][:, :PEDG].T)
    out = np.ascontiguousarray(res.results[0]["g_out"].T)  # [16, 1]
    n_full = np.ascontiguousarray(np.concatenate(n_parts, axis=0))
    e_full = np.ascontiguousarray(np.concatenate(e_parts, axis=0))
    return out, n_full, e_full


# revision 3
# speedup vs baseline: 1.2018x; 1.2018x over previous
"""BayesianGNN forward pass on 8 Trainium2 NeuronCores (Bass/Tile).

The reference network's jraph aggregations are dead code, so nodes/edges/
globals are independent per-row MLP chains.  The Bayesian weight noise is
deterministic (EPS_SEED=42), so effective dense weights are computed on the
host and adjacent layer pairs with no ReLU between them are folded:

  node/edge chain: x[64] -> (We@W00) relu -> W01 relu -> (W02@W10) relu
                   -> W11 relu -> W12                      (5 matmuls/row)
  globals chain:   g[64] -> 6x[128,128] relu -> readout [128,1]  (N=16)

Device layout: activations feature-major [feat, rows]; rows data-parallel
over 8 cores; each 512-row half-chunk is an independent pipeline:
matmul (float32r) -> PSUM [128,512] -> fused bias(+relu) epilogue
(ScalarE/VectorE 7:6 weighted round-robin) -> SBUF -> DMA out.
"""
import os
import sys

if '/opt/trn_rl_repo' not in sys.path:
    sys.path.insert(0, '/opt/trn_rl_repo')

import numpy as np

import concourse.tile as tile
from concourse import bacc, mybir, bass_utils

f32 = mybir.dt.float32
f32r = mybir.dt.float32r
AF = mybir.ActivationFunctionType
ALU = mybir.AluOpType

N_NODES = 100_000
N_EDGES = 400_000
N_CORES = 8
PN = N_NODES // N_CORES            # node rows per core
PEDG = N_EDGES // N_CORES          # edge rows per core
DCH = 1024
RN = ((PN + DCH - 1) // DCH) * DCH     # 13312
RE = ((PEDG + DCH - 1) // DCH) * DCH   # 50176
NDCH_N = RN // DCH
NDCH_E = RE // DCH

MODE = os.environ.get("GNN_MODE", "f32r")   # "f32r" | "f32"


# ----------------------------------------------------------------- weights
def _pick_eps_backend(nodes):
    """The environment's default PRNG impl (rbg) is platform-dependent.
    The reference generated its inputs and weight noise on some backend;
    find a backend whose rbg stream reproduces `nodes` so our eps matches
    the reference's."""
    import jax

    probe = np.asarray(nodes[:2], np.float32)

    def gen(dev):
        with jax.default_device(dev):
            ks = jax.random.split(jax.random.key(0), 12)
            cand = jax.random.normal(ks[0], (N_NODES, 64), 'float32')
            return np.asarray(cand[:2], np.float32)

    default_dev = jax.devices()[0]
    try:
        if np.allclose(gen(default_dev), probe, atol=1e-6):
            return default_dev
    except Exception:
        pass
    try:
        cpu = jax.devices('cpu')[0]
        if np.allclose(gen(cpu), probe, atol=1e-6):
            return cpu
    except Exception:
        pass
    return default_dev


def fold_params(params, nodes):
    import jax
    import jax.numpy as jnp

    dev = _pick_eps_backend(nodes)

    with jax.default_device(dev):
        def eff_layers(layers, key):
            ks = jax.random.split(key, len(layers))
            out = []
            for lyr, k in zip(layers, ks):
                wm, wr = jnp.asarray(lyr['wm']), jnp.asarray(lyr['wr'])
                bm, br = jnp.asarray(lyr['bm']), jnp.asarray(lyr['br'])
                w = wm + jax.random.normal(k, wm.shape) * jnp.log1p(jnp.exp(wr))
                b = bm + jax.random.normal(k, bm.shape) * jnp.log1p(jnp.exp(br))
                out.append((np.asarray(w, np.float64),
                            np.asarray(b, np.float64)))
            return out

        keys = jax.random.split(jax.random.key(42), 4)
        n_mlps = [eff_layers(params['node_mlp'][i], keys[i]) for i in range(2)]
        e_mlps = [eff_layers(params['edge_mlp'][i], keys[i]) for i in range(2)]
        g_mlps = [eff_layers(params['global_mlp'][i], keys[i])
                  for i in range(2)]
        ro = eff_layers(params['readout'], keys[3])

    def dense(p):
        return (np.asarray(p['w'], np.float64), np.asarray(p['b'], np.float64))

    We, be = dense(params['node_embed'])
    Ee, ee = dense(params['edge_embed'])
    Ge, ge = dense(params['global_embed'])

    def chain(embed_w, embed_b, mlps):
        (w00, b00), (w01, b01), (w02, b02) = mlps[0]
        (w10, b10), (w11, b11), (w12, b12) = mlps[1]
        A = [embed_w @ w00, w01, w02 @ w10, w11, w12]
        c = [embed_b @ w00 + b00, b01, b02 @ w10 + b10, b11, b12]
        return A, c

    An, cn = chain(We, be, n_mlps)
    Ae, ce = chain(Ee, ee, e_mlps)
    (gw00, gb00), (gw01, gb01), (gw02, gb02) = g_mlps[0]
    (gw10, gb10), (gw11, gb11), (gw12, gb12) = g_mlps[1]
    (r0, rb0), (r1, rb1), (r2, rb2) = ro
    Ag = [Ge @ gw00, gw01, gw02 @ gw10, gw11, gw12 @ r0, r1, r2]
    cg = [ge @ gw00 + gb00, gb01, gb02 @ gw10 + gb10, gb11,
          gb12 @ r0 + rb0, rb1, rb2]

    tof = lambda a: np.ascontiguousarray(np.asarray(a, np.float32))
    return {
        'An': [tof(a) for a in An], 'cn': [tof(a) for a in cn],
        'Ae': [tof(a) for a in Ae], 'ce': [tof(a) for a in ce],
        'Ag': [tof(a) for a in Ag], 'cg': [tof(a) for a in cg],
    }


# ----------------------------------------------------------------- program
_PROG = {}


def build_program(mode):
    if mode in _PROG:
        return _PROG[mode]
    mm_dt = f32r if mode == "f32r" else f32

    nc = bacc.Bacc("TRN2", debug=False)
    nodes_in = nc.dram_tensor("nodes_in", [128, RN // 2], mm_dt,
                              kind="ExternalInput").ap()
    edges_in = nc.dram_tensor("edges_in", [128, RE // 2], mm_dt,
                              kind="ExternalInput").ap()
    g_in = nc.dram_tensor("g_in", [64, 16], f32, kind="ExternalInput").ap()
    nodes_out = nc.dram_tensor("nodes_out", [128, RN], f32,
                               kind="ExternalOutput").ap()
    edges_out = nc.dram_tensor("edges_out", [128, RE], f32,
                               kind="ExternalOutput").ap()
    g_out = nc.dram_tensor("g_out", [1, 16], f32, kind="ExternalOutput").ap()

    wdr = {}
    for pre in ("n", "e"):
        wdr[pre] = {
            'w': [nc.dram_tensor(f"w{pre}{i}", [128, 128], mm_dt,
                                 kind="ExternalInput").ap() for i in range(5)],
            'b': [nc.dram_tensor(f"b{pre}{i}", [128, 1], f32,
                                 kind="ExternalInput").ap() for i in range(5)],
        }
    wg_dr, bg_dr = [], []
    for i in range(7):
        kshape = 64 if i == 0 else 128
        mshape = 1 if i == 6 else 128
        wg_dr.append(nc.dram_tensor(f"wg{i}", [kshape, mshape], f32,
                                    kind="ExternalInput").ap())
        bg_dr.append(nc.dram_tensor(f"bg{i}", [mshape, 1], f32,
                                    kind="ExternalInput").ap())

    with tile.TileContext(nc) as tc:
        with tc.tile_pool(name="wpool", bufs=1) as wpool, \
             tc.tile_pool(name="ipool", bufs=12) as ipool, \
             tc.tile_pool(name="hpool", bufs=32) as hpool, \
             tc.tile_pool(name="opool", bufs=8) as opool, \
             tc.tile_pool(name="pspool", bufs=8, space="PSUM") as pspool:

            wt, bt = {}, {}
            for pre in ("n", "e"):
                wt[pre], bt[pre] = [], []
                for i in range(5):
                    w = wpool.tile([128, 128], mm_dt, tag=f"w{pre}{i}",
                                   name=f"w{pre}{i}_sb")
                    nc.sync.dma_start(out=w[:], in_=wdr[pre]['w'][i])
                    wt[pre].append(w)
                    b = wpool.tile([128, 1], f32, tag=f"b{pre}{i}",
                                   name=f"b{pre}{i}_sb")
                    nc.sync.dma_start(out=b[:], in_=wdr[pre]['b'][i])
                    bt[pre].append(b)
            wgt, bgt = [], []
            for i in range(7):
                kshape = 64 if i == 0 else 128
                mshape = 1 if i == 6 else 128
                w = wpool.tile([kshape, mshape], f32, tag=f"wg{i}",
                               name=f"wg{i}_sb")
                nc.sync.dma_start(out=w[:], in_=wg_dr[i])
                wgt.append(w)
                b = wpool.tile([mshape, 1], f32, tag=f"bg{i}",
                               name=f"bg{i}_sb")
                nc.sync.dma_start(out=b[:], in_=bg_dr[i])
                bgt.append(b)

            # ---- globals chain (fp32, N=16, replicated on every core) ----
            gt = wpool.tile([64, 16], f32, tag="gt", name="gt_sb")
            nc.sync.dma_start(out=gt[:], in_=g_in)
            cur = gt
            for i in range(7):
                mshape = 1 if i == 6 else 128
                ps = pspool.tile([mshape, 16], f32, tag="ps",
                                 name=f"gps{i}")
                nc.tensor.matmul(ps[:], wgt[i][:], cur[:], start=True,
                                 stop=True)
                if i < 6:
                    nxt = wpool.tile([128, 16], f32, tag=f"gs{i % 2}",
                                     name=f"gs{i}")
                    nc.scalar.activation(nxt[:], ps[:], AF.Relu,
                                         bias=bgt[i][:])
                else:
                    nxt = wpool.tile([1, 16], f32, tag="gout_sb",
                                     name="gout_sb")
                    nc.scalar.activation(nxt[:], ps[:], AF.Identity,
                                         bias=bgt[i][:])
                cur = nxt
            nc.gpsimd.dma_start(out=g_out, in_=cur[:])

            # ---- node/edge main loops ----
            # Each 512-column half of a dchunk is an independent chain
            # (features on partitions).  Blocks of BLK dchunks = 2*BLK
            # half-chains, layer-major: the PE sees 2*BLK independent
            # matmuls per layer; [128,512] epilogues are distributed
            # ACT:DVE by a 7:6 weighted round-robin.
            BLK = 6
            EPAT = [0, 1, 0, 1, 0, 1, 0, 0, 1, 0, 1, 0, 1]  # 7 ACT, 6 DVE
            ectr = [0]

            def do_chain(x_in, x_out, W, B, ndch):
                for b0 in range(0, ndch, BLK):
                    blk = range(b0, min(b0 + BLK, ndch))
                    nhc = 2 * len(blk)
                    its = []
                    for d in blk:
                        it = ipool.tile([128, 512], mm_dt, tag="in",
                                        name=f"in{d}")
                        nc.sync.dma_start(out=it[:],
                                          in_=x_in[:, 512 * d:512 * (d + 1)])
                        its.append(it)
                    ots = [opool.tile([128, 1024], f32, tag="ot",
                                      name=f"ot{j}") for j in range(len(blk))]
                    cur = [None] * nhc
                    for L in range(5):
                        pss = [pspool.tile([128, 512], f32, tag="ps",
                                           name=f"ps_{L}_{q}")
                               for q in range(nhc)]
                        for q in range(nhc):
                            j, h = q // 2, q % 2
                            if L == 0:
                                lhsT = W[0][64 * h:64 * (h + 1), :]
                                rhs = its[j][64 * h:64 * (h + 1), :]
                            else:
                                lhsT = W[L][:]
                                rhs = cur[q][:]
                            nc.tensor.matmul(pss[q][:], lhsT, rhs,
                                             start=True, stop=True)
                        for q in range(nhc):
                            j, h = q // 2, q % 2
                            ps = pss[q]
                            on_act = EPAT[ectr[0] % len(EPAT)] == 0
                            ectr[0] += 1
                            if L < 4:
                                hb = hpool.tile([128, 512], mm_dt, tag="h",
                                                name=f"h_{L}_{q}")
                                if on_act:
                                    nc.scalar.activation(hb[:], ps[:],
                                                         AF.Relu,
                                                         bias=B[L][:])
                                else:
                                    nc.vector.tensor_scalar(
                                        out=hb[:], in0=ps[:], scalar1=B[L][:],
                                        scalar2=0.0, op0=ALU.add, op1=ALU.max)
                                cur[q] = hb
                            else:
                                dst = ots[j][:, 512 * h:512 * (h + 1)]
                                if on_act:
                                    nc.scalar.activation(dst, ps[:],
                                                         AF.Identity,
                                                         bias=B[4][:])
                                else:
                                    nc.vector.tensor_scalar_add(
                                        out=dst, in0=ps[:], scalar1=B[4][:])
                    for j, d in enumerate(blk):
                        nc.gpsimd.dma_start(
                            out=x_out[:, DCH * d:DCH * (d + 1)], in_=ots[j][:])

            do_chain(nodes_in, nodes_out, wt["n"], bt["n"], NDCH_N)
            do_chain(edges_in, edges_out, wt["e"], bt["e"], NDCH_E)
    nc.compile()
    _PROG[mode] = nc
    return nc


# ------------------------------------------------------------------ kernel
def _pack(xT, R):
    """[64, R] feature-major -> [128, R//2] two-row-group packed layout."""
    d = R // DCH
    return np.ascontiguousarray(
        xT.reshape(64, d, 2, 512).transpose(2, 0, 1, 3).reshape(128, R // 2))


LAST_RESULT = {}


def kernel(nodes, edges, globals_, params, senders, receivers):
    mode = MODE
    nc = build_program(mode)

    nodes = np.asarray(nodes, np.float32)
    edges = np.asarray(edges, np.float32)
    globals_ = np.asarray(globals_, np.float32)

    fw = fold_params(params, nodes)

    weight_map = {"g_in": np.ascontiguousarray(globals_.T)}
    for pre, Akey, ckey in (("n", "An", "cn"), ("e", "Ae", "ce")):
        A, c = fw[Akey], fw[ckey]
        # embed lhsT duplicated so both 64-partition halves carry it
        weight_map[f"w{pre}0"] = np.ascontiguousarray(
            np.concatenate([A[0], A[0]], axis=0))
        for i in range(1, 5):
            weight_map[f"w{pre}{i}"] = A[i]
        for i in range(5):
            weight_map[f"b{pre}{i}"] = np.ascontiguousarray(
                c[i].reshape(-1, 1))
    for i in range(7):
        weight_map[f"wg{i}"] = fw['Ag'][i]
        weight_map[f"bg{i}"] = np.ascontiguousarray(fw['cg'][i].reshape(-1, 1))

    in_maps = []
    for cidx in range(N_CORES):
        nT = np.zeros((64, RN), np.float32)
        nT[:, :PN] = nodes[cidx * PN:(cidx + 1) * PN].T
        eT = np.zeros((64, RE), np.float32)
        eT[:, :PEDG] = edges[cidx * PEDG:(cidx + 1) * PEDG].T
        m = dict(weight_map)
        m["nodes_in"] = _pack(nT, RN)
        m["edges_in"] = _pack(eT, RE)
        in_maps.append(m)

    trace = os.environ.get("GNN_TRACE") == "1"
    res = None
    last_err = None
    for _attempt in range(3):
        try:
            res = bass_utils.run_bass_kernel_spmd(
                nc, in_maps, core_ids=list(range(N_CORES)), trace=trace)
            break
        except Exception as e:  # transient device errors observed; retry
            last_err = e
    if res is None:
        raise last_err
    LAST_RESULT["res"] = res

    n_parts, e_parts = [], []
    for cidx in range(N_CORES):
        r = res.results[cidx]
        n_parts.append(r["nodes_out"][:, :PN].T)
        e_parts.append(r["edges_out"][:, :PEDG].T)
    out = np.ascontiguousarray(res.results[0]["g_out"].T)  # [16, 1]
    n_full = np.ascontiguousarray(np.concatenate(n_parts, axis=0))
    e_full = np.ascontiguousarray(np.concatenate(e_parts, axis=0))
    return out, n_full, e_full


# revision 4
# speedup vs baseline: 1.2115x; 1.0081x over previous
"""BayesianGNN forward pass on 8 Trainium2 NeuronCores (Bass/Tile).

The reference network's jraph aggregations are dead code, so nodes/edges/
globals are independent per-row MLP chains.  The Bayesian weight noise is
deterministic (EPS_SEED=42), so effective dense weights are computed on the
host and adjacent layer pairs with no ReLU between them are folded:

  node/edge chain: x[64] -> (We@W00) relu -> W01 relu -> (W02@W10) relu
                   -> W11 relu -> W12                      (5 matmuls/row)
  globals chain:   g[64] -> 6x[128,128] relu -> readout [128,1]  (N=16)

Device layout: activations feature-major [feat, rows]; rows data-parallel
over 8 cores; each 512-row half-chunk is an independent pipeline:
matmul (float32r) -> PSUM [128,512] -> fused bias(+relu) epilogue
(ScalarE/VectorE 7:6 weighted round-robin) -> SBUF -> DMA out.
"""
import os
import sys

if '/opt/trn_rl_repo' not in sys.path:
    sys.path.insert(0, '/opt/trn_rl_repo')

import numpy as np

import concourse.tile as tile
from concourse import bacc, mybir, bass_utils

f32 = mybir.dt.float32
f32r = mybir.dt.float32r
AF = mybir.ActivationFunctionType
ALU = mybir.AluOpType

N_NODES = 100_000
N_EDGES = 400_000
N_CORES = 8
PN = N_NODES // N_CORES            # node rows per core
PEDG = N_EDGES // N_CORES          # edge rows per core
DCH = 1024
RN = ((PN + DCH - 1) // DCH) * DCH     # 13312
RE = ((PEDG + DCH - 1) // DCH) * DCH   # 50176
NDCH_N = RN // DCH
NDCH_E = RE // DCH

MODE = os.environ.get("GNN_MODE", "f32r")   # "f32r" | "f32"


# ----------------------------------------------------------------- weights
def _pick_eps_backend(nodes):
    """The environment's default PRNG impl (rbg) is platform-dependent.
    The reference generated its inputs and weight noise on some backend;
    find a backend whose rbg stream reproduces `nodes` so our eps matches
    the reference's."""
    import jax

    probe = np.asarray(nodes[:2], np.float32)

    def gen(dev):
        with jax.default_device(dev):
            ks = jax.random.split(jax.random.key(0), 12)
            cand = jax.random.normal(ks[0], (N_NODES, 64), 'float32')
            return np.asarray(cand[:2], np.float32)

    default_dev = jax.devices()[0]
    try:
        if np.allclose(gen(default_dev), probe, atol=1e-6):
            return default_dev
    except Exception:
        pass
    try:
        cpu = jax.devices('cpu')[0]
        if np.allclose(gen(cpu), probe, atol=1e-6):
            return cpu
    except Exception:
        pass
    return default_dev


def fold_params(params, nodes):
    import jax
    import jax.numpy as jnp

    dev = _pick_eps_backend(nodes)

    with jax.default_device(dev):
        def eff_layers(layers, key):
            ks = jax.random.split(key, len(layers))
            out = []
            for lyr, k in zip(layers, ks):
                wm, wr = jnp.asarray(lyr['wm']), jnp.asarray(lyr['wr'])
                bm, br = jnp.asarray(lyr['bm']), jnp.asarray(lyr['br'])
                w = wm + jax.random.normal(k, wm.shape) * jnp.log1p(jnp.exp(wr))
                b = bm + jax.random.normal(k, bm.shape) * jnp.log1p(jnp.exp(br))
                out.append((np.asarray(w, np.float64),
                            np.asarray(b, np.float64)))
            return out

        keys = jax.random.split(jax.random.key(42), 4)
        n_mlps = [eff_layers(params['node_mlp'][i], keys[i]) for i in range(2)]
        e_mlps = [eff_layers(params['edge_mlp'][i], keys[i]) for i in range(2)]
        g_mlps = [eff_layers(params['global_mlp'][i], keys[i])
                  for i in range(2)]
        ro = eff_layers(params['readout'], keys[3])

    def dense(p):
        return (np.asarray(p['w'], np.float64), np.asarray(p['b'], np.float64))

    We, be = dense(params['node_embed'])
    Ee, ee = dense(params['edge_embed'])
    Ge, ge = dense(params['global_embed'])

    def chain(embed_w, embed_b, mlps):
        (w00, b00), (w01, b01), (w02, b02) = mlps[0]
        (w10, b10), (w11, b11), (w12, b12) = mlps[1]
        A = [embed_w @ w00, w01, w02 @ w10, w11, w12]
        c = [embed_b @ w00 + b00, b01, b02 @ w10 + b10, b11, b12]
        return A, c

    An, cn = chain(We, be, n_mlps)
    Ae, ce = chain(Ee, ee, e_mlps)
    (gw00, gb00), (gw01, gb01), (gw02, gb02) = g_mlps[0]
    (gw10, gb10), (gw11, gb11), (gw12, gb12) = g_mlps[1]
    (r0, rb0), (r1, rb1), (r2, rb2) = ro
    Ag = [Ge @ gw00, gw01, gw02 @ gw10, gw11, gw12 @ r0, r1, r2]
    cg = [ge @ gw00 + gb00, gb01, gb02 @ gw10 + gb10, gb11,
          gb12 @ r0 + rb0, rb1, rb2]

    tof = lambda a: np.ascontiguousarray(np.asarray(a, np.float32))
    return {
        'An': [tof(a) for a in An], 'cn': [tof(a) for a in cn],
        'Ae': [tof(a) for a in Ae], 'ce': [tof(a) for a in ce],
        'Ag': [tof(a) for a in Ag], 'cg': [tof(a) for a in cg],
    }


# ----------------------------------------------------------------- program
_PROG = {}


def build_program(mode):
    if mode in _PROG:
        return _PROG[mode]
    mm_dt = f32r if mode == "f32r" else f32

    nc = bacc.Bacc("TRN2", debug=False)
    nodes_in = nc.dram_tensor("nodes_in", [128, RN // 2], mm_dt,
                              kind="ExternalInput").ap()
    edges_in = nc.dram_tensor("edges_in", [128, RE // 2], mm_dt,
                              kind="ExternalInput").ap()
    g_in = nc.dram_tensor("g_in", [64, 16], f32, kind="ExternalInput").ap()
    nodes_out = nc.dram_tensor("nodes_out", [128, RN], f32,
                               kind="ExternalOutput").ap()
    edges_out = nc.dram_tensor("edges_out", [128, RE], f32,
                               kind="ExternalOutput").ap()
    g_out = nc.dram_tensor("g_out", [1, 16], f32, kind="ExternalOutput").ap()

    wdr = {}
    for pre in ("n", "e"):
        wdr[pre] = {
            'w': [nc.dram_tensor(f"w{pre}{i}", [128, 128], mm_dt,
                                 kind="ExternalInput").ap() for i in range(5)],
            'b': [nc.dram_tensor(f"b{pre}{i}", [128, 1], f32,
                                 kind="ExternalInput").ap() for i in range(5)],
        }
    wg_dr, bg_dr = [], []
    for i in range(7):
        kshape = 64 if i == 0 else 128
        mshape = 1 if i == 6 else 128
        wg_dr.append(nc.dram_tensor(f"wg{i}", [kshape, mshape], f32,
                                    kind="ExternalInput").ap())
        bg_dr.append(nc.dram_tensor(f"bg{i}", [mshape, 1], f32,
                                    kind="ExternalInput").ap())

    with tile.TileContext(nc) as tc:
        with tc.tile_pool(name="wpool", bufs=1) as wpool, \
             tc.tile_pool(name="ipool", bufs=3) as ipool, \
             tc.tile_pool(name="hpool", bufs=32) as hpool, \
             tc.tile_pool(name="opool", bufs=2) as opool, \
             tc.tile_pool(name="pspool", bufs=8, space="PSUM") as pspool:

            wt, bt = {}, {}
            for pre in ("n", "e"):
                wt[pre], bt[pre] = [], []
                for i in range(5):
                    w = wpool.tile([128, 128], mm_dt, tag=f"w{pre}{i}",
                                   name=f"w{pre}{i}_sb")
                    nc.sync.dma_start(out=w[:], in_=wdr[pre]['w'][i])
                    wt[pre].append(w)
                    b = wpool.tile([128, 1], f32, tag=f"b{pre}{i}",
                                   name=f"b{pre}{i}_sb")
                    nc.sync.dma_start(out=b[:], in_=wdr[pre]['b'][i])
                    bt[pre].append(b)
            wgt, bgt = [], []
            for i in range(7):
                kshape = 64 if i == 0 else 128
                mshape = 1 if i == 6 else 128
                w = wpool.tile([kshape, mshape], f32, tag=f"wg{i}",
                               name=f"wg{i}_sb")
                nc.sync.dma_start(out=w[:], in_=wg_dr[i])
                wgt.append(w)
                b = wpool.tile([mshape, 1], f32, tag=f"bg{i}",
                               name=f"bg{i}_sb")
                nc.sync.dma_start(out=b[:], in_=bg_dr[i])
                bgt.append(b)

            # ---- globals chain (fp32, N=16, replicated on every core) ----
            gt = wpool.tile([64, 16], f32, tag="gt", name="gt_sb")
            nc.sync.dma_start(out=gt[:], in_=g_in)
            cur = gt
            for i in range(7):
                mshape = 1 if i == 6 else 128
                ps = pspool.tile([mshape, 16], f32, tag="ps",
                                 name=f"gps{i}")
                nc.tensor.matmul(ps[:], wgt[i][:], cur[:], start=True,
                                 stop=True)
                if i < 6:
                    nxt = wpool.tile([128, 16], f32, tag=f"gs{i % 2}",
                                     name=f"gs{i}")
                    nc.scalar.activation(nxt[:], ps[:], AF.Relu,
                                         bias=bgt[i][:])
                else:
                    nxt = wpool.tile([1, 16], f32, tag="gout_sb",
                                     name="gout_sb")
                    nc.scalar.activation(nxt[:], ps[:], AF.Identity,
                                         bias=bgt[i][:])
                cur = nxt
            nc.gpsimd.dma_start(out=g_out, in_=cur[:])

            # ---- node/edge main loops ----
            # Each 512-column half of a dchunk is an independent chain
            # (features on partitions).  Blocks of BLK dchunks = 2*BLK
            # half-chains, layer-major: the PE sees 2*BLK independent
            # matmuls per layer; [128,512] epilogues are distributed
            # ACT:DVE by a 7:6 weighted round-robin.
            BLK = 6
            EPAT = [0, 1, 0, 1, 0, 1, 0, 0, 1, 0, 1, 0, 1]  # 7 ACT, 6 DVE
            ectr = [0]

            def do_chain(x_in, x_out, W, B, ndch):
                for b0 in range(0, ndch, BLK):
                    blk = range(b0, min(b0 + BLK, ndch))
                    nb = len(blk)
                    nhc = 2 * nb
                    big_in = ipool.tile([128, 512 * nb], mm_dt, tag="in",
                                        name=f"in{b0}")
                    nc.sync.dma_start(out=big_in[:],
                                      in_=x_in[:, 512 * b0:512 * (b0 + nb)])
                    its = [big_in[:, 512 * j:512 * (j + 1)]
                           for j in range(nb)]
                    big_ot = opool.tile([128, DCH * nb], f32, tag="ot",
                                        name=f"ot{b0}")
                    cur = [None] * nhc
                    for L in range(5):
                        pss = [pspool.tile([128, 512], f32, tag="ps",
                                           name=f"ps_{L}_{q}")
                               for q in range(nhc)]
                        for q in range(nhc):
                            j, h = q // 2, q % 2
                            if L == 0:
                                lhsT = W[0][64 * h:64 * (h + 1), :]
                                rhs = big_in[64 * h:64 * (h + 1),
                                             512 * j:512 * (j + 1)]
                            else:
                                lhsT = W[L][:]
                                rhs = cur[q][:]
                            nc.tensor.matmul(pss[q][:], lhsT, rhs,
                                             start=True, stop=True)
                        for q in range(nhc):
                            j, h = q // 2, q % 2
                            ps = pss[q]
                            on_act = EPAT[ectr[0] % len(EPAT)] == 0
                            ectr[0] += 1
                            if L < 4:
                                hb = hpool.tile([128, 512], mm_dt, tag="h",
                                                name=f"h_{L}_{q}")
                                if on_act:
                                    nc.scalar.activation(hb[:], ps[:],
                                                         AF.Relu,
                                                         bias=B[L][:])
                                else:
                                    nc.vector.tensor_scalar(
                                        out=hb[:], in0=ps[:], scalar1=B[L][:],
                                        scalar2=0.0, op0=ALU.add, op1=ALU.max)
                                cur[q] = hb
                            else:
                                dst = big_ot[:, DCH * j + 512 * h:
                                             DCH * j + 512 * (h + 1)]
                                if on_act:
                                    nc.scalar.activation(dst, ps[:],
                                                         AF.Identity,
                                                         bias=B[4][:])
                                else:
                                    nc.vector.tensor_scalar_add(
                                        out=dst, in0=ps[:], scalar1=B[4][:])
                    nc.gpsimd.dma_start(
                        out=x_out[:, DCH * b0:DCH * (b0 + nb)],
                        in_=big_ot[:])

            do_chain(nodes_in, nodes_out, wt["n"], bt["n"], NDCH_N)
            do_chain(edges_in, edges_out, wt["e"], bt["e"], NDCH_E)
    nc.compile()
    _PROG[mode] = nc
    return nc


# ------------------------------------------------------------------ kernel
def _pack(xT, R):
    """[64, R] feature-major -> [128, R//2] two-row-group packed layout."""
    d = R // DCH
    return np.ascontiguousarray(
        xT.reshape(64, d, 2, 512).transpose(2, 0, 1, 3).reshape(128, R // 2))


LAST_RESULT = {}


def kernel(nodes, edges, globals_, params, senders, receivers):
    mode = MODE
    nc = build_program(mode)

    nodes = np.asarray(nodes, np.float32)
    edges = np.asarray(edges, np.float32)
    globals_ = np.asarray(globals_, np.float32)

    fw = fold_params(params, nodes)

    weight_map = {"g_in": np.ascontiguousarray(globals_.T)}
    for pre, Akey, ckey in (("n", "An", "cn"), ("e", "Ae", "ce")):
        A, c = fw[Akey], fw[ckey]
        # embed lhsT duplicated so both 64-partition halves carry it
        weight_map[f"w{pre}0"] = np.ascontiguousarray(
            np.concatenate([A[0], A[0]], axis=0))
        for i in range(1, 5):
            weight_map[f"w{pre}{i}"] = A[i]
        for i in range(5):
            weight_map[f"b{pre}{i}"] = np.ascontiguousarray(
                c[i].reshape(-1, 1))
    for i in range(7):
        weight_map[f"wg{i}"] = fw['Ag'][i]
        weight_map[f"bg{i}"] = np.ascontiguousarray(fw['cg'][i].reshape(-1, 1))

    in_maps = []
    for cidx in range(N_CORES):
        nT = np.zeros((64, RN), np.float32)
        nT[:, :PN] = nodes[cidx * PN:(cidx + 1) * PN].T
        eT = np.zeros((64, RE), np.float32)
        eT[:, :PEDG] = edges[cidx * PEDG:(cidx + 1) * PEDG].T
        m = dict(weight_map)
        m["nodes_in"] = _pack(nT, RN)
        m["edges_in"] = _pack(eT, RE)
        in_maps.append(m)

    trace = os.environ.get("GNN_TRACE") == "1"
    res = None
    last_err = None
    for _attempt in range(3):
        try:
            res = bass_utils.run_bass_kernel_spmd(
                nc, in_maps, core_ids=list(range(N_CORES)), trace=trace)
            break
        except Exception as e:  # transient device errors observed; retry
            last_err = e
    if res is None:
        raise last_err
    LAST_RESULT["res"] = res

    n_parts, e_parts = [], []
    for cidx in range(N_CORES):
        r = res.results[cidx]
        n_parts.append(r["nodes_out"][:, :PN].T)
        e_parts.append(r["edges_out"][:, :PEDG].T)
    out = np.ascontiguousarray(res.results[0]["g_out"].T)  # [16, 1]
    n_full = np.ascontiguousarray(np.concatenate(n_parts, axis=0))
    e_full = np.ascontiguousarray(np.concatenate(e_parts, axis=0))
    return out, n_full, e_full
